# revision 5
# baseline (speedup 1.0000x reference)
"""Multi-head self-attention (RoPE) Trainium2 Bass kernel.

Shards batch (B=8) across 8 NeuronCores, one batch element per core.
Per core: fused qkv projection (fp16 matmuls), RoPE, flash-style attention
(scores row-tiled per head pair, exp on ACT with fused scale+mask-bias,
AV row-tiled by k-parity with a fused ones-column producing softmax
denominators), reciprocal-normalize, output projection.

The rep body sits inside a tc.For_i hardware loop: the NEFF replays the
same static instruction stream per rep, so steady-state per-rep cost is
dynamic execution only (~150us, at the PE moving-column roofline) instead
of paying ~25-70us/instruction static dispatch for unrolled reps. Phase-2
emission interleaves pair p's score groups with pair p-1's AV groups at
4-matmul granularity so the PE queue never stalls on the ACT exp of the
single-buffered scores PSUM.
"""
import os
import sys

# The kernel needs the 8 axon-tunneled NeuronCores visible to jax; a
# JAX_PLATFORMS=cpu pin (used by some harnesses for the reference) would
# hide them. Clear it before jax initializes through the concourse imports.
os.environ.pop("JAX_PLATFORMS", None)

sys.path.insert(0, "/opt/trn_rl_repo")

_REPS = int(os.environ.get("KREPS", "1"))
_PH = int(os.environ.get("KPHASES", "3"))

import numpy as np
from contextlib import ExitStack

import concourse.bass as bass
import concourse.tile as tile
from concourse import bacc, mybir
from concourse.tile import add_dep_helper

f32 = mybir.dt.float32
f16 = mybir.dt.float16
AF = mybir.ActivationFunctionType
ALU = mybir.AluOpType

B, L, DIM = 8, 1024, 512
NH, HD = 8, 64
SCALE = HD ** -0.5
NCORES = 8


def _build_nc():
    nc = bacc.Bacc("TRN2", target_bir_lowering=False, debug=False, enable_asserts=False)

    xT = nc.dram_tensor("xT", (DIM, L), f16, kind="ExternalInput")
    wq = nc.dram_tensor("wq", (DIM, 2 * DIM), f16, kind="ExternalInput")  # Q|K cols
    wv = nc.dram_tensor("wv", (DIM, DIM), f16, kind="ExternalInput")      # V cols
    wp = nc.dram_tensor("wp", (DIM, DIM), f16, kind="ExternalInput")
    cosT = nc.dram_tensor("cosT", (128, L), f16, kind="ExternalInput")
    sinT = nc.dram_tensor("sinT", (128, L), f16, kind="ExternalInput")
    bias = nc.dram_tensor("bias", (128, 8), f32, kind="ExternalInput")
    y = nc.dram_tensor("y", (L, DIM), f32, kind="ExternalOutput")

    with ExitStack() as ctx:
        tc = ctx.enter_context(tile.TileContext(nc))
        cst = ctx.enter_context(tc.tile_pool(name="cst", bufs=1))
        sc = ctx.enter_context(tc.tile_pool(name="sc", bufs=3))
        pTp = ctx.enter_context(tc.tile_pool(name="pTp", bufs=10))
        nrm = ctx.enter_context(tc.tile_pool(name="nrm", bufs=2))
        ysb = ctx.enter_context(tc.tile_pool(name="ysb", bufs=1))

        # ---- load inputs ----
        xT_all = cst.tile([128, 4 * L], f16, name="t", tag="xTall")
        wq_all = cst.tile([128, 4 * 2 * DIM], f16, name="t", tag="wqall")
        wv_all = cst.tile([128, 4 * DIM], f16, name="t", tag="wvall")
        wp_all = cst.tile([128, 4 * DIM], f16, name="t", tag="wpall")
        for big, dram, w in ((xT_all, xT, L), (wq_all, wq, 2 * DIM),
                             (wv_all, wv, DIM), (wp_all, wp, DIM)):
            nc.sync.dma_start(
                big[:].rearrange("p (kc w) -> p kc w", kc=4),
                dram[:].rearrange("(kc p) w -> p kc w", p=128))
        xT_sb = [xT_all[:, i * L:(i + 1) * L] for i in range(4)]
        wq_sb = [wq_all[:, i * 2 * DIM:(i + 1) * 2 * DIM] for i in range(4)]
        wv_sb = [wv_all[:, i * DIM:(i + 1) * DIM] for i in range(4)]
        wp_sb = [wp_all[:, i * DIM:(i + 1) * DIM] for i in range(4)]
        cos_sb = cst.tile([128, L], f16, name="t", tag="cos")
        sin_sb = cst.tile([128, L], f16, name="t", tag="sin")
        bias_sb = cst.tile([128, 8], f32, name="t", tag="bias")
        nc.sync.dma_start(cos_sb[:], cosT[:])
        nc.sync.dma_start(sin_sb[:], sinT[:])
        nc.sync.dma_start(bias_sb[:], bias[:])

        qkT = [cst.tile([64, L], f16, name="t", tag=f"qkT{m}") for m in range(16)]
        vaug = [cst.tile([128, NH * 128], f16, name="t", tag=f"vaug{i}") for i in range(8)]
        outT = [cst.tile([128, L], f16, name="t", tag=f"outT{c}") for c in range(4)]

        def emit_body(rep):
            untiled1 = []
            tiled = []
            # ---------- phase 1: qkv projection + RoPE ----------
            with tc.tile_pool(name=f"qkps{rep}", bufs=2, space="PSUM") as qk_ps, \
                 tc.tile_pool(name=f"vps{rep}", bufs=2, space="PSUM") as v_ps:
                for m in range(8):
                    ps = qk_ps.tile([128, L], f32, name="t", tag="qkps")
                    for kc in range(4):
                        for qb in range(2):
                            mm = nc.tensor.matmul(
                                ps[:, qb * 512:(qb + 1) * 512],
                                wq_sb[kc][:, m * 128:(m + 1) * 128],
                                xT_sb[kc][:, qb * 512:(qb + 1) * 512],
                                start=(kc == 0), stop=(kc == 3))
                            untiled1.append(mm)
                    qc = sc.tile([128, L], f16, name="t", tag="qc")
                    nc.vector.tensor_copy(qc[:], ps[:])
                    sw = sc.tile([128, L], f16, name="t", tag="sw")
                    for (do, so) in ((0, 32), (32, 0), (64, 96), (96, 64)):
                        nc.vector.tensor_copy(sw[do:do + 32, :], qc[so:so + 32, :])
                    q1 = sc.tile([128, L], f16, name="t", tag="q1")
                    nc.vector.tensor_mul(q1[:], qc[:], cos_sb[:])
                    q2 = sc.tile([128, L], f16, name="t", tag="q2")
                    nc.vector.tensor_mul(q2[:], sw[:], sin_sb[:])
                    nc.vector.tensor_add(qkT[2 * m][:], q1[0:64, :], q2[0:64, :])
                    nc.vector.tensor_add(qkT[2 * m + 1][:], q1[64:128, :], q2[64:128, :])

                for lb in range(8):
                    vps = v_ps.tile([128, DIM], f32, name="t", tag="vps")
                    for kc in range(4):
                        mm = nc.tensor.matmul(
                            vps[:],
                            xT_sb[kc][:, lb * 128:(lb + 1) * 128],
                            wv_sb[kc][:],
                            start=(kc == 0), stop=(kc == 3))
                        untiled1.append(mm)
                    ones_ap = vaug[lb][:].rearrange("p (h c) -> p h c", h=NH)[:, :, 64:128]
                    nc.vector.memset(ones_ap, 1.0)
                    out_ap = vaug[lb][:].rearrange("p (h c) -> p h c", h=NH)[:, :, 0:64]
                    in_ap = vps[:].rearrange("p (h c) -> p h c", h=NH)
                    nc.vector.tensor_copy(out_ap, in_ap)

            # ---------- phase 2: attention (row-tiled 64x128) ----------
            # Emission is interleaved at group granularity: while pair p's
            # scores wait on the ACT exp (s_ps is single-buffered), the PE
            # queue holds AV matmuls of pair p-1, so it never idles.
            with tc.tile_pool(name=f"sps{rep}", bufs=1, space="PSUM") as s_ps, \
                 tc.tile_pool(name=f"avps{rep}", bufs=2, space="PSUM") as av_ps:

                def score_groups(p):
                    """Yield 8 generators, each emitting one (t, kb2) scores
                    group (4 matmuls + exp) and yielding the pt tile."""
                    pts = {}

                    def grp(t, kb2):
                        h = 2 * p + t
                        QTc, KTc = qkT[h], qkT[8 + h]
                        s = s_ps.tile([128, 2048], f32, name="t", tag="s")
                        for half in range(2):
                            kb = kb2 * 2 + half
                            for qb in range(2):
                                mm = nc.tensor.matmul(
                                    s[:, half * 1024 + qb * 512:
                                       half * 1024 + (qb + 1) * 512],
                                    KTc[:, kb * 128:(kb + 1) * 128],
                                    QTc[:, qb * 512:(qb + 1) * 512],
                                    start=True, stop=True)
                                tiled.append(mm)
                        pt = pTp.tile([128, 2048], f16, name="t", tag="pT")
                        nc.scalar.activation(pt[:], s[:], AF.Exp,
                                             bias=bias_sb[:, 2 * kb2:2 * kb2 + 1],
                                             scale=SCALE)
                        pts[(t, kb2)] = pt

                    return pts, [(t, kb2) for t in range(2) for kb2 in range(4)], grp

                def av_groups(p, pts):
                    """Return state + 8 group emitters of 4 AV matmuls each,
                    plus a finisher list for the norm chains."""
                    Xs = {}

                    def grp(t, kc2):
                        h = 2 * p + t
                        if kc2 == 0:
                            Xs[t] = av_ps.tile([128, L], f32, name="t", tag="avX")
                        X = Xs[t]
                        for kc in (2 * kc2, 2 * kc2 + 1):
                            pt = pts[(t, kc // 2)]
                            off = (kc % 2) * 1024
                            va = vaug[kc][:, h * 128:(h + 1) * 128]
                            for qb in range(2):
                                q0, q1_ = off + qb * 512, off + (qb + 1) * 512
                                mm = nc.tensor.matmul(
                                    X[:, qb * 512:(qb + 1) * 512], va, pt[:, q0:q1_],
                                    start=(kc == 0), stop=(kc == 7))
                                tiled.append(mm)

                    def norm(t):
                        h = 2 * p + t
                        X = Xs[t]
                        D = nrm.tile([64, L], f32, name="t", tag="D")
                        nc.vector.tensor_copy(D[:], X[64:128, :])
                        R = nrm.tile([64, L], f32, name="t", tag="R")
                        nc.vector.reciprocal_approx_fast(R[:], D[:])
                        if t == 0:
                            nc.vector.tensor_mul(outT[p][0:64, :], X[0:64, :], R[:])
                        else:
                            tmp = nrm.tile([64, L], f16, name="t", tag="tmp")
                            nc.vector.tensor_mul(tmp[:], X[0:64, :], R[:])
                            nc.sync.dma_start(outT[p][64:128, :], tmp[:])

                    return [(t, kc2) for t in range(2) for kc2 in range(4)], grp, norm

                prev = None
                for p in range(4 if _PH >= 2 else 0):
                    pts, skeys, sgrp = score_groups(p)
                    if prev is not None and _PH >= 3:
                        pp_, akeys, agrp, anorm = prev
                        for (sk, ak) in zip(skeys, akeys):
                            sgrp(*sk)
                            agrp(*ak)
                            if ak == (0, 3):
                                anorm(0)
                        anorm(1)
                    else:
                        for sk in skeys:
                            sgrp(*sk)
                    if _PH >= 3:
                        akeys, agrp, anorm = av_groups(p, pts)
                        prev = (p, akeys, agrp, anorm)
                if prev is not None and _PH >= 3:
                    pp_, akeys, agrp, anorm = prev
                    for ak in akeys:
                        agrp(*ak)
                        if ak == (0, 3):
                            anorm(0)
                    anorm(1)

            # ---------- phase 3: output projection ----------
            with tc.tile_pool(name=f"yps{rep}", bufs=2, space="PSUM") as y_ps:
                yall = ysb.tile([128, 8 * DIM], f32, name="t", tag="yall")
                if _PH < 3:
                    nc.vector.memset(yall[:], 0.0)
                for lb in range(8 if _PH >= 3 else 0):
                    yp = y_ps.tile([128, DIM], f32, name="t", tag="yps")
                    for c in range(4):
                        mm = nc.tensor.matmul(
                            yp[:],
                            outT[c][:, lb * 128:(lb + 1) * 128],
                            wp_sb[c][:],
                            start=(c == 0), stop=(c == 3))
                    nc.vector.tensor_copy(yall[:, lb * DIM:(lb + 1) * DIM], yp[:])
                nc.sync.dma_start(
                    y[:].rearrange("(lb p) d -> p lb d", p=128),
                    yall[:].rearrange("p (lb d) -> p lb d", lb=8))

        # Hardware loop over reps: the NEFF's static instruction stream is
        # emitted once and replayed, so per-rep cost is dynamic-execution
        # only (static instruction fetch/dispatch dominates unrolled reps).
        with tc.For_i(0, _REPS):
            emit_body(0)

    nc.compile()
    return nc


def _rope_tables():
    inv_freq = 1.0 / (10000.0 ** (np.arange(0, HD, 2, dtype=np.float32) / HD))
    t = np.arange(L, dtype=np.float32)
    freqs = np.outer(t, inv_freq)                      # (L, 32)
    emb = np.concatenate([freqs, freqs], axis=-1)      # (L, 64)
    cos = np.cos(emb).T                                # (64, L)
    sin = np.sin(emb).T                                # (64, L)
    sign = np.where(np.arange(HD) < HD // 2, -1.0, 1.0)[:, None].astype(np.float32)
    sin_s = sin * sign
    cosT = np.tile(cos, (2, 1)).astype(np.float16)     # (128, L)
    sinT = np.tile(sin_s, (2, 1)).astype(np.float16)   # (128, L)
    return cosT, sinT


_NC = None


def _get_nc():
    global _NC
    if _NC is None:
        _NC = _build_nc()
    return _NC


def kernel(x, mask, w_qkv, w_proj):
    x = np.asarray(x, dtype=np.float32)
    mask = np.asarray(mask)
    w_qkv = np.asarray(w_qkv, dtype=np.float32)
    w_proj = np.asarray(w_proj, dtype=np.float32)

    nc = _get_nc()
    cosT, sinT = _rope_tables()

    wq = np.ascontiguousarray(w_qkv[:, :2 * DIM]).astype(np.float16)
    wv = np.ascontiguousarray(w_qkv[:, 2 * DIM:]).astype(np.float16)
    wp = w_proj.astype(np.float16)

    in_maps = []
    for b in range(NCORES):
        xTb = np.ascontiguousarray(x[b].T).astype(np.float16)      # (512, 1024)
        bias_b = np.where(mask[b].reshape(8, 128).T, 0.0, -1e9).astype(np.float32)
        in_maps.append({
            "xT": xTb, "wq": wq, "wv": wv, "wp": wp,
            "cosT": cosT, "sinT": sinT, "bias": bias_b,
        })

    from concourse.bass_utils import run_bass_kernel_spmd
    res = run_bass_kernel_spmd(nc, in_maps, core_ids=list(range(NCORES)))
    out = np.stack([res.results[c]["y"] for c in range(NCORES)], axis=0)
    return out.astype(np.float32)



# revision 6
# speedup vs baseline: 1.1663x; 1.1663x over previous
"""Multi-head self-attention (RoPE) Trainium2 Bass kernel.

Shards batch (B=8) across 8 NeuronCores, one batch element per core.
Per core: fused qkv projection (fp16 matmuls), RoPE, flash-style attention
(scores row-tiled per head pair, exp on ACT with fused scale+mask-bias,
AV with a fused ones-column block producing softmax denominators),
reciprocal-normalize, output projection.

Performance structure:
- The rep body sits inside a tc.For_i hardware loop: the NEFF replays one
  static instruction stream per rep, so steady-state per-rep cost is
  dynamic execution only (~PE moving-column roofline) instead of paying
  ~25-70us/instruction static dispatch for unrolled reps.
- Phase-2 emission interleaves pair p's score groups with pair p-1's AV
  groups at 4-matmul granularity so the PE queue never stalls on the ACT
  exp of the single-buffered scores PSUM.
- V tiles carry the ones-block on opposite partition halves for even/odd
  heads, so a head-pair's denominators land on complementary partition
  halves and normalization needs no cross-partition DMA.
- RoPE and PSUM->SBUF evacuation run as wide (2048-col) DVE ops over
  m-block pairs; the ones-block memset is hoisted out of the rep loop;
  the output DMA is a contiguous 128x16KB descriptor copy (host undoes
  the block-row layout).
"""
import os
import sys

# The kernel needs the 8 axon-tunneled NeuronCores visible to jax; a
# JAX_PLATFORMS=cpu pin (used by some harnesses for the reference) would
# hide them. Clear it before jax initializes through the concourse imports.
os.environ.pop("JAX_PLATFORMS", None)

sys.path.insert(0, "/opt/trn_rl_repo")

_REPS = int(os.environ.get("KREPS", "1"))
_PH = int(os.environ.get("KPHASES", "3"))

import numpy as np
from contextlib import ExitStack

import concourse.bass as bass
import concourse.tile as tile
from concourse import bacc, mybir
from concourse.tile import add_dep_helper

f32 = mybir.dt.float32
f16 = mybir.dt.float16
AF = mybir.ActivationFunctionType
ALU = mybir.AluOpType

B, L, DIM = 8, 1024, 512
NH, HD = 8, 64
SCALE = HD ** -0.5
NCORES = 8

# qkT slot layout: within a phase-1 group g (m-blocks 2g, 2g+1), the two
# wide RoPE adds write heads (4g, 4g+2) rows 0:64 and (4g+1, 4g+3) rows
# 64:128 as contiguous 2048-col spans. K-heads sit 8 slots later.
_WMAP = {0: 0, 2: 1, 1: 2, 3: 3}


def _slot_q(h):
    return (h // 4) * 4 + _WMAP[h % 4]


def _slot_k(h):
    return 8 + _slot_q(h)


def _build_nc():
    nc = bacc.Bacc("TRN2", target_bir_lowering=False, debug=False, enable_asserts=False)

    xT = nc.dram_tensor("xT", (DIM, L), f16, kind="ExternalInput")
    wq = nc.dram_tensor("wq", (DIM, 2 * DIM), f16, kind="ExternalInput")  # Q|K cols
    wv = nc.dram_tensor("wv", (DIM, DIM), f16, kind="ExternalInput")      # V cols
    wp = nc.dram_tensor("wp", (DIM, DIM), f16, kind="ExternalInput")
    cosT = nc.dram_tensor("cosT", (128, 2 * L), f16, kind="ExternalInput")
    sinT = nc.dram_tensor("sinT", (128, 2 * L), f16, kind="ExternalInput")
    bias = nc.dram_tensor("bias", (128, 8), f32, kind="ExternalInput")
    # y holds the block-row layout (p, lb*DIM+d) = out[lb*128+p, d]; the
    # host reshapes back. Keeps the per-rep output DMA fully contiguous.
    y = nc.dram_tensor("y", (128, 8 * DIM), f32, kind="ExternalOutput")

    with ExitStack() as ctx:
        tc = ctx.enter_context(tile.TileContext(nc))
        cst = ctx.enter_context(tc.tile_pool(name="cst", bufs=1))
        sc = ctx.enter_context(tc.tile_pool(name="sc", bufs=2))
        pTp = ctx.enter_context(tc.tile_pool(name="pTp", bufs=10))
        nrm = ctx.enter_context(tc.tile_pool(name="nrm", bufs=2))
        ysb = ctx.enter_context(tc.tile_pool(name="ysb", bufs=1))

        # ---- load inputs ----
        xT_all = cst.tile([128, 4 * L], f16, name="t", tag="xTall")
        wq_all = cst.tile([128, 4 * 2 * DIM], f16, name="t", tag="wqall")
        wv_all = cst.tile([128, 4 * DIM], f16, name="t", tag="wvall")
        wp_all = cst.tile([128, 4 * DIM], f16, name="t", tag="wpall")
        for big, dram, w in ((xT_all, xT, L), (wq_all, wq, 2 * DIM),
                             (wv_all, wv, DIM), (wp_all, wp, DIM)):
            nc.sync.dma_start(
                big[:].rearrange("p (kc w) -> p kc w", kc=4),
                dram[:].rearrange("(kc p) w -> p kc w", p=128))
        xT_sb = [xT_all[:, i * L:(i + 1) * L] for i in range(4)]
        wq_sb = [wq_all[:, i * 2 * DIM:(i + 1) * 2 * DIM] for i in range(4)]
        wv_sb = [wv_all[:, i * DIM:(i + 1) * DIM] for i in range(4)]
        wp_sb = [wp_all[:, i * DIM:(i + 1) * DIM] for i in range(4)]
        cos_sb = cst.tile([128, 2 * L], f16, name="t", tag="cos")
        sin_sb = cst.tile([128, 2 * L], f16, name="t", tag="sin")
        bias_sb = cst.tile([128, 8], f32, name="t", tag="bias")
        nc.sync.dma_start(cos_sb[:], cosT[:])
        nc.sync.dma_start(sin_sb[:], sinT[:])
        nc.sync.dma_start(bias_sb[:], bias[:])

        # Q|K heads, one 16-slot tile; slot order per _slot_q/_slot_k.
        qkT = cst.tile([64, 16 * L], f16, name="t", tag="qkT")

        def qk_ap(slot):
            return qkT[:, slot * L:(slot + 1) * L]

        # V (augmented): per lb block of 1024 cols, head h at h*128..h*128+128.
        # Even h: [V(0:64) | ones(64:128)]; odd h: [ones(0:64) | V(64:128)].
        # The single pre-loop memset seeds the ones; in-loop V copies only
        # touch the V columns.
        vaug = cst.tile([128, 8 * NH * 128], f16, name="t", tag="vaug")
        nc.vector.memset(vaug[:], 1.0)

        outT = [cst.tile([128, L], f16, name="t", tag=f"outT{c}") for c in range(4)]

        def emit_body(rep):
            # ---------- phase 1a: Q,K projection + RoPE (wide, m-pairs) ----
            with tc.tile_pool(name=f"qkps{rep}", bufs=2, space="PSUM") as qk_ps:
                for g in range(4):
                    ps = qk_ps.tile([128, 2048], f32, name="t", tag="qkps")
                    for mi in range(2):
                        m = 2 * g + mi
                        for kc in range(4):
                            for qb in range(2):
                                nc.tensor.matmul(
                                    ps[:, mi * 1024 + qb * 512:
                                       mi * 1024 + (qb + 1) * 512],
                                    wq_sb[kc][:, m * 128:(m + 1) * 128],
                                    xT_sb[kc][:, qb * 512:(qb + 1) * 512],
                                    start=(kc == 0), stop=(kc == 3))
                    qc = sc.tile([128, 2048], f16, name="t", tag="qc")
                    nc.vector.tensor_copy(qc[:], ps[:])
                    sw = sc.tile([128, 2048], f16, name="t", tag="sw")
                    for (do, so) in ((0, 32), (32, 0), (64, 96), (96, 64)):
                        nc.vector.tensor_copy(sw[do:do + 32, :], qc[so:so + 32, :])
                    q1 = sc.tile([128, 2048], f16, name="t", tag="q1")
                    nc.vector.tensor_mul(q1[:], qc[:], cos_sb[:])
                    q2 = sc.tile([128, 2048], f16, name="t", tag="q2")
                    nc.vector.tensor_mul(q2[:], sw[:], sin_sb[:])
                    # heads (4g, 4g+2) rows 0:64 -> slots 4s+0,1;
                    # heads (4g+1, 4g+3) rows 64:128 -> slots 4s+2,3
                    base = (g % 2) * 4 + (8 if g >= 2 else 0)
                    nc.vector.tensor_add(qkT[:, base * L:(base + 2) * L],
                                         q1[0:64, :], q2[0:64, :])
                    nc.vector.tensor_add(qkT[:, (base + 2) * L:(base + 4) * L],
                                         q1[64:128, :], q2[64:128, :])

            # ---------- phase 1b: V projection (wide, lb-quads) ----------
            with tc.tile_pool(name=f"vps{rep}", bufs=2, space="PSUM") as v_ps:
                for lq in range(2):
                    vps = v_ps.tile([128, 2048], f32, name="t", tag="vps")
                    for li in range(4):
                        lb = 4 * lq + li
                        for kc in range(4):
                            nc.tensor.matmul(
                                vps[:, li * 512:(li + 1) * 512],
                                xT_sb[kc][:, lb * 128:(lb + 1) * 128],
                                wv_sb[kc][:],
                                start=(kc == 0), stop=(kc == 3))
                    for li in range(4):
                        lb = 4 * lq + li
                        src = vps[:, li * 512:(li + 1) * 512].rearrange(
                            "p (g c2) -> p g c2", g=4)
                        dst = vaug[:, lb * 1024:(lb + 1) * 1024].rearrange(
                            "p (g c4) -> p g c4", g=4)
                        nc.vector.tensor_copy(dst[:, :, 0:64], src[:, :, 0:64])
                        nc.vector.tensor_copy(dst[:, :, 192:256], src[:, :, 64:128])

            # ---------- phase 2: attention (row-tiled 64x128) ----------
            # Interleaved: pair p's scores fill the PE queue between pair
            # p-1's AV groups, so PE never idles on the ACT exp.
            with tc.tile_pool(name=f"sps{rep}", bufs=1, space="PSUM") as s_ps, \
                 tc.tile_pool(name=f"avps{rep}", bufs=2, space="PSUM") as av_ps:

                def score_groups(p):
                    pts = {}

                    def grp(t, kb2):
                        h = 2 * p + t
                        QTc, KTc = qk_ap(_slot_q(h)), qk_ap(_slot_k(h))
                        s = s_ps.tile([128, 2048], f32, name="t", tag="s")
                        for half in range(2):
                            kb = kb2 * 2 + half
                            for qb in range(2):
                                nc.tensor.matmul(
                                    s[:, half * 1024 + qb * 512:
                                       half * 1024 + (qb + 1) * 512],
                                    KTc[:, kb * 128:(kb + 1) * 128],
                                    QTc[:, qb * 512:(qb + 1) * 512],
                                    start=True, stop=True)
                        pt = pTp.tile([128, 2048], f16, name="t", tag="pT")
                        nc.scalar.activation(pt[:], s[:], AF.Exp,
                                             bias=bias_sb[:, 2 * kb2:2 * kb2 + 1],
                                             scale=SCALE)
                        pts[(t, kb2)] = pt

                    return pts, [(t, kb2) for t in range(2) for kb2 in range(4)], grp

                def av_groups(p, pts):
                    Xs = {}

                    def grp(t, kc2):
                        h = 2 * p + t
                        if kc2 == 0:
                            Xs[t] = av_ps.tile([128, L], f32, name="t", tag="avX")
                        X = Xs[t]
                        for kc in (2 * kc2, 2 * kc2 + 1):
                            pt = pts[(t, kc // 2)]
                            off = (kc % 2) * 1024
                            va = vaug[:, kc * 1024 + h * 128:kc * 1024 + (h + 1) * 128]
                            for qb in range(2):
                                q0, q1_ = off + qb * 512, off + (qb + 1) * 512
                                nc.tensor.matmul(
                                    X[:, qb * 512:(qb + 1) * 512], va, pt[:, q0:q1_],
                                    start=(kc == 0), stop=(kc == 7))

                    def norm_a():
                        # even head's denominators (rows 64:128) -> D rows 0:64
                        D = nrm.tile([128, L], f32, name="t", tag="D")
                        nc.vector.tensor_copy(D[0:64, :], Xs[0][64:128, :])
                        Xs["D"] = D

                    def norm_b():
                        D = Xs["D"]
                        # odd head's denominators (rows 0:64) -> D rows 64:128
                        nc.vector.tensor_copy(D[64:128, :], Xs[1][0:64, :])
                        R = nrm.tile([128, L], f32, name="t", tag="R")
                        nc.vector.reciprocal_approx_fast(R[:], D[:])
                        nc.vector.tensor_mul(outT[p][0:64, :], Xs[0][0:64, :],
                                             R[0:64, :])
                        nc.vector.tensor_mul(outT[p][64:128, :], Xs[1][64:128, :],
                                             R[64:128, :])

                    return ([(t, kc2) for t in range(2) for kc2 in range(4)],
                            grp, norm_a, norm_b)

                def drain_av(prev):
                    pp_, akeys, agrp, na, nb = prev
                    for ak in akeys:
                        agrp(*ak)
                        if ak == (0, 3):
                            na()
                    nb()

                prev = None
                for p in range(4 if _PH >= 2 else 0):
                    pts, skeys, sgrp = score_groups(p)
                    if prev is not None and _PH >= 3:
                        pp_, akeys, agrp, na, nb = prev
                        for (sk, ak) in zip(skeys, akeys):
                            sgrp(*sk)
                            agrp(*ak)
                            if ak == (0, 3):
                                na()
                        nb()
                    else:
                        for sk in skeys:
                            sgrp(*sk)
                    if _PH >= 3:
                        akeys, agrp, na, nb = av_groups(p, pts)
                        prev = (p, akeys, agrp, na, nb)
                if prev is not None and _PH >= 3:
                    drain_av(prev)

            # ---------- phase 3: output projection (wide, lb-quads) ------
            with tc.tile_pool(name=f"yps{rep}", bufs=2, space="PSUM") as y_ps:
                yall = ysb.tile([128, 8 * DIM], f32, name="t", tag="yall")
                if _PH < 3:
                    nc.vector.memset(yall[:], 0.0)
                for lq in range(2 if _PH >= 3 else 0):
                    yp = y_ps.tile([128, 2048], f32, name="t", tag="yps")
                    for li in range(4):
                        lb = 4 * lq + li
                        for c in range(4):
                            nc.tensor.matmul(
                                yp[:, li * 512:(li + 1) * 512],
                                outT[c][:, lb * 128:(lb + 1) * 128],
                                wp_sb[c][:],
                                start=(c == 0), stop=(c == 3))
                    nc.vector.tensor_copy(yall[:, lq * 2048:(lq + 1) * 2048], yp[:])
                nc.sync.dma_start(y[:], yall[:])

        # Hardware loop over reps: the static instruction stream is emitted
        # once and replayed, so per-rep cost is dynamic-execution only.
        with tc.For_i(0, _REPS):
            emit_body(0)

    nc.compile()
    return nc


def _rope_tables():
    inv_freq = 1.0 / (10000.0 ** (np.arange(0, HD, 2, dtype=np.float32) / HD))
    t = np.arange(L, dtype=np.float32)
    freqs = np.outer(t, inv_freq)                      # (L, 32)
    emb = np.concatenate([freqs, freqs], axis=-1)      # (L, 64)
    cos = np.cos(emb).T                                # (64, L)
    sin = np.sin(emb).T                                # (64, L)
    sign = np.where(np.arange(HD) < HD // 2, -1.0, 1.0)[:, None].astype(np.float32)
    sin_s = sin * sign
    cosT = np.tile(cos, (2, 2)).astype(np.float16)     # (128, 2L)
    sinT = np.tile(sin_s, (2, 2)).astype(np.float16)   # (128, 2L)
    return cosT, sinT


_NC = None


def _get_nc():
    global _NC
    if _NC is None:
        _NC = _build_nc()
    return _NC


def kernel(x, mask, w_qkv, w_proj):
    x = np.asarray(x, dtype=np.float32)
    mask = np.asarray(mask)
    w_qkv = np.asarray(w_qkv, dtype=np.float32)
    w_proj = np.asarray(w_proj, dtype=np.float32)

    nc = _get_nc()
    cosT, sinT = _rope_tables()

    wq = np.ascontiguousarray(w_qkv[:, :2 * DIM]).astype(np.float16)
    wv = np.ascontiguousarray(w_qkv[:, 2 * DIM:]).astype(np.float16)
    wp = w_proj.astype(np.float16)

    in_maps = []
    for b in range(NCORES):
        xTb = np.ascontiguousarray(x[b].T).astype(np.float16)      # (512, 1024)
        bias_b = np.where(mask[b].reshape(8, 128).T, 0.0, -1e9).astype(np.float32)
        in_maps.append({
            "xT": xTb, "wq": wq, "wv": wv, "wp": wp,
            "cosT": cosT, "sinT": sinT, "bias": bias_b,
        })

    from concourse.bass_utils import run_bass_kernel_spmd
    res = run_bass_kernel_spmd(nc, in_maps, core_ids=list(range(NCORES)))
    out = np.stack([
        res.results[c]["y"].reshape(128, 8, DIM).transpose(1, 0, 2).reshape(L, DIM)
        for c in range(NCORES)
    ], axis=0)
    return out.astype(np.float32)


# revision 10
# speedup vs baseline: 1.2279x; 1.0528x over previous
"""Multi-head self-attention (RoPE) Trainium2 Bass kernel.

Shards batch (B=8) across 8 NeuronCores, one batch element per core.
Per core: fused qkv projection (fp16 matmuls), RoPE, flash-style attention
(scores row-tiled per head pair, exp on ACT with fused scale+mask-bias,
AV with a fused ones-column block producing softmax denominators),
reciprocal-normalize, output projection.

Performance structure:
- The rep body sits inside a tc.For_i hardware loop: the NEFF replays one
  static instruction stream per rep, so steady-state per-rep cost is
  dynamic execution only (~PE moving-column roofline) instead of paying
  ~25-70us/instruction static dispatch for unrolled reps.
- Phase-2 emission interleaves pair p's score groups with pair p-1's AV
  groups at 4-matmul granularity so the PE queue never stalls on the ACT
  exp of the single-buffered scores PSUM.
- V tiles carry the ones-block on opposite partition halves for even/odd
  heads, so a head-pair's denominators land on complementary partition
  halves and normalization needs no cross-partition DMA.
- RoPE and PSUM->SBUF evacuation run as wide (2048-col) DVE ops over
  m-block pairs; the ones-block memset is hoisted out of the rep loop;
  the output DMA is a contiguous 128x16KB descriptor copy (host undoes
  the block-row layout).
"""
import os
import sys

# The kernel needs the 8 axon-tunneled NeuronCores visible to jax; a
# JAX_PLATFORMS=cpu pin (used by some harnesses for the reference) would
# hide them. Clear it before jax initializes through the concourse imports.
os.environ.pop("JAX_PLATFORMS", None)

sys.path.insert(0, "/opt/trn_rl_repo")

_REPS = int(os.environ.get("KREPS", "1"))
_PH = int(os.environ.get("KPHASES", "3"))

import numpy as np
from contextlib import ExitStack

import concourse.bass as bass
import concourse.tile as tile
from concourse import bacc, mybir
from concourse.tile import add_dep_helper

f32 = mybir.dt.float32
f16 = mybir.dt.float16
AF = mybir.ActivationFunctionType
ALU = mybir.AluOpType

B, L, DIM = 8, 1024, 512
NH, HD = 8, 64
SCALE = HD ** -0.5
NCORES = 8

# qkT slot layout: within a phase-1 group g (m-blocks 2g, 2g+1), the two
# wide RoPE adds write heads (4g, 4g+2) rows 0:64 and (4g+1, 4g+3) rows
# 64:128 as contiguous 2048-col spans. K-heads sit 8 slots later.
_WMAP = {0: 0, 2: 1, 1: 2, 3: 3}


def _slot_q(h):
    return (h // 4) * 4 + _WMAP[h % 4]


def _slot_k(h):
    return 8 + _slot_q(h)


def _build_nc():
    nc = bacc.Bacc("TRN2", target_bir_lowering=False, debug=False, enable_asserts=False)

    xT = nc.dram_tensor("xT", (DIM, L), f16, kind="ExternalInput")
    wq = nc.dram_tensor("wq", (DIM, 2 * DIM), f16, kind="ExternalInput")  # Q|K cols
    wv = nc.dram_tensor("wv", (DIM, DIM), f16, kind="ExternalInput")      # V cols
    wp = nc.dram_tensor("wp", (DIM, DIM), f16, kind="ExternalInput")
    cosT = nc.dram_tensor("cosT", (128, 2 * L), f16, kind="ExternalInput")
    sinT = nc.dram_tensor("sinT", (128, 2 * L), f16, kind="ExternalInput")
    bias = nc.dram_tensor("bias", (128, 8), f32, kind="ExternalInput")
    # y holds the block-row layout (p, lb*DIM+d) = out[lb*128+p, d]; the
    # host reshapes back. Keeps the per-rep output DMA fully contiguous.
    y = nc.dram_tensor("y", (128, 8 * DIM), f32, kind="ExternalOutput")

    with ExitStack() as ctx:
        tc = ctx.enter_context(tile.TileContext(nc))
        cst = ctx.enter_context(tc.tile_pool(name="cst", bufs=1))
        sc = ctx.enter_context(tc.tile_pool(name="sc", bufs=2))
        pTp = ctx.enter_context(tc.tile_pool(name="pTp", bufs=20))
        nrm = ctx.enter_context(tc.tile_pool(name="nrm", bufs=2))
        ysb = ctx.enter_context(tc.tile_pool(name="ysb", bufs=1))

        # ---- load inputs ----
        xT_all = cst.tile([128, 4 * L], f16, name="t", tag="xTall")
        wq_all = cst.tile([128, 4 * 2 * DIM], f16, name="t", tag="wqall")
        wv_all = cst.tile([128, 4 * DIM], f16, name="t", tag="wvall")
        wp_all = cst.tile([128, 4 * DIM], f16, name="t", tag="wpall")
        for big, dram, w in ((xT_all, xT, L), (wq_all, wq, 2 * DIM),
                             (wv_all, wv, DIM), (wp_all, wp, DIM)):
            nc.sync.dma_start(
                big[:].rearrange("p (kc w) -> p kc w", kc=4),
                dram[:].rearrange("(kc p) w -> p kc w", p=128))
        xT_sb = [xT_all[:, i * L:(i + 1) * L] for i in range(4)]
        wq_sb = [wq_all[:, i * 2 * DIM:(i + 1) * 2 * DIM] for i in range(4)]
        wv_sb = [wv_all[:, i * DIM:(i + 1) * DIM] for i in range(4)]
        wp_sb = [wp_all[:, i * DIM:(i + 1) * DIM] for i in range(4)]
        cos_sb = cst.tile([128, 2 * L], f16, name="t", tag="cos")
        sin_sb = cst.tile([128, 2 * L], f16, name="t", tag="sin")
        bias_sb = cst.tile([128, 8], f32, name="t", tag="bias")
        nc.sync.dma_start(cos_sb[:], cosT[:])
        nc.sync.dma_start(sin_sb[:], sinT[:])
        nc.sync.dma_start(bias_sb[:], bias[:])

        # Q|K heads, one 16-slot tile; slot order per _slot_q/_slot_k.
        qkT = cst.tile([64, 16 * L], f16, name="t", tag="qkT")

        def qk_ap(slot):
            return qkT[:, slot * L:(slot + 1) * L]

        # V (augmented): per lb block of 1024 cols, head h at h*128..h*128+128.
        # Even h: [V(0:64) | ones(64:128)]; odd h: [ones(0:64) | V(64:128)].
        # The single pre-loop memset seeds the ones; in-loop V copies only
        # touch the V columns.
        vaug = cst.tile([128, 8 * NH * 128], f16, name="t", tag="vaug")
        nc.vector.memset(vaug[:], 1.0)

        outT = [cst.tile([128, L], f16, name="t", tag=f"outT{c}") for c in range(4)]

        def emit_body(rep):
            # ---------- phase 1a: Q,K projection + RoPE (wide, m-pairs) ----
            with tc.tile_pool(name=f"qkps{rep}", bufs=2, space="PSUM") as qk_ps:
                for g in range(4):
                    ps = qk_ps.tile([128, 2048], f32, name="t", tag="qkps")
                    for mi in range(2):
                        m = 2 * g + mi
                        for kc in range(4):
                            for qb in range(2):
                                nc.tensor.matmul(
                                    ps[:, mi * 1024 + qb * 512:
                                       mi * 1024 + (qb + 1) * 512],
                                    wq_sb[kc][:, m * 128:(m + 1) * 128],
                                    xT_sb[kc][:, qb * 512:(qb + 1) * 512],
                                    start=(kc == 0), stop=(kc == 3))
                    qc = sc.tile([128, 2048], f16, name="t", tag="qc")
                    nc.vector.tensor_copy(qc[:], ps[:])
                    sw = sc.tile([128, 2048], f16, name="t", tag="sw")
                    for (do, so) in ((0, 32), (32, 0), (64, 96), (96, 64)):
                        nc.vector.tensor_copy(sw[do:do + 32, :], qc[so:so + 32, :])
                    q1 = sc.tile([128, 2048], f16, name="t", tag="q1")
                    nc.vector.tensor_mul(q1[:], qc[:], cos_sb[:])
                    q2 = sc.tile([128, 2048], f16, name="t", tag="q2")
                    nc.vector.tensor_mul(q2[:], sw[:], sin_sb[:])
                    # heads (4g, 4g+2) rows 0:64 -> slots 4s+0,1;
                    # heads (4g+1, 4g+3) rows 64:128 -> slots 4s+2,3
                    base = (g % 2) * 4 + (8 if g >= 2 else 0)
                    nc.vector.tensor_add(qkT[:, base * L:(base + 2) * L],
                                         q1[0:64, :], q2[0:64, :])
                    nc.vector.tensor_add(qkT[:, (base + 2) * L:(base + 4) * L],
                                         q1[64:128, :], q2[64:128, :])

            # ---------- phase 1b: V projection (wide, lb-quads) ----------
            with tc.tile_pool(name=f"vps{rep}", bufs=2, space="PSUM") as v_ps:
                for lq in range(2):
                    vps = v_ps.tile([128, 2048], f32, name="t", tag="vps")
                    for li in range(4):
                        lb = 4 * lq + li
                        for kc in range(4):
                            nc.tensor.matmul(
                                vps[:, li * 512:(li + 1) * 512],
                                xT_sb[kc][:, lb * 128:(lb + 1) * 128],
                                wv_sb[kc][:],
                                start=(kc == 0), stop=(kc == 3))
                    for li in range(4):
                        lb = 4 * lq + li
                        src = vps[:, li * 512:(li + 1) * 512].rearrange(
                            "p (g c2) -> p g c2", g=4)
                        dst = vaug[:, lb * 1024:(lb + 1) * 1024].rearrange(
                            "p (g c4) -> p g c4", g=4)
                        nc.vector.tensor_copy(dst[:, :, 0:64], src[:, :, 0:64])
                        nc.vector.tensor_copy(dst[:, :, 192:256], src[:, :, 64:128])

            # ---------- phase 2: attention (row-tiled 64x128) ----------
            # Interleaved: pair p's scores fill the PE queue between pair
            # p-1's AV groups, so PE never idles on the ACT exp.
            with tc.tile_pool(name=f"sps{rep}", bufs=2, space="PSUM") as s_ps, \
                 tc.tile_pool(name=f"avps{rep}", bufs=2, space="PSUM") as av_ps:

                def score_groups(p):
                    pts = {}

                    def grp(t, kb):
                        h = 2 * p + t
                        QTc, KTc = qk_ap(_slot_q(h)), qk_ap(_slot_k(h))
                        s = s_ps.tile([128, 1024], f32, name="t", tag="s")
                        for qb in range(2):
                            nc.tensor.matmul(
                                s[:, qb * 512:(qb + 1) * 512],
                                KTc[:, kb * 128:(kb + 1) * 128],
                                QTc[:, qb * 512:(qb + 1) * 512],
                                start=True, stop=True)
                        pt = pTp.tile([128, 1024], f16, name="t", tag="pT")
                        nc.scalar.activation(pt[:], s[:], AF.Exp,
                                             bias=bias_sb[:, kb:kb + 1],
                                             scale=SCALE)
                        pts[(t, kb)] = pt

                    return pts, [(t, kb) for t in range(2) for kb in range(8)], grp

                def av_groups(p, pts):
                    Xs = {}

                    def grp(t, kc2):
                        h = 2 * p + t
                        if kc2 == 0:
                            Xs[t] = av_ps.tile([128, L], f32, name="t", tag="avX")
                        X = Xs[t]
                        for kc in (2 * kc2, 2 * kc2 + 1):
                            pt = pts[(t, kc)]
                            va = vaug[:, kc * 1024 + h * 128:kc * 1024 + (h + 1) * 128]
                            for qb in range(2):
                                nc.tensor.matmul(
                                    X[:, qb * 512:(qb + 1) * 512], va,
                                    pt[:, qb * 512:(qb + 1) * 512],
                                    start=(kc == 0), stop=(kc == 7))

                    def norm_a():
                        # even head's denominators (rows 64:128) -> D rows 0:64
                        D = nrm.tile([128, L], f32, name="t", tag="D")
                        nc.vector.tensor_copy(D[0:64, :], Xs[0][64:128, :])
                        Xs["D"] = D

                    def norm_b():
                        D = Xs["D"]
                        # odd head's denominators (rows 0:64) -> D rows 64:128
                        nc.vector.tensor_copy(D[64:128, :], Xs[1][0:64, :])
                        R = nrm.tile([128, L], f32, name="t", tag="R")
                        nc.vector.reciprocal_approx_fast(R[:], D[:])
                        nc.vector.tensor_mul(outT[p][0:64, :], Xs[0][0:64, :],
                                             R[0:64, :])
                        nc.vector.tensor_mul(outT[p][64:128, :], Xs[1][64:128, :],
                                             R[64:128, :])

                    return ([(t, kc2) for t in range(2) for kc2 in range(4)],
                            grp, norm_a, norm_b)

                def drain_av(prev):
                    pp_, akeys, agrp, na, nb = prev
                    for ak in akeys:
                        agrp(*ak)
                        if ak == (0, 3):
                            na()
                    nb()

                prev = None
                for p in range(4 if _PH >= 2 else 0):
                    pts, skeys, sgrp = score_groups(p)
                    if prev is not None and _PH >= 3:
                        pp_, akeys, agrp, na, nb = prev
                        for i, ak in enumerate(akeys):
                            sgrp(*skeys[2 * i])
                            sgrp(*skeys[2 * i + 1])
                            agrp(*ak)
                            if ak == (0, 3):
                                na()
                        nb()
                    else:
                        for sk in skeys:
                            sgrp(*sk)
                    if _PH >= 3:
                        akeys, agrp, na, nb = av_groups(p, pts)
                        prev = (p, akeys, agrp, na, nb)
                if prev is not None and _PH >= 3:
                    drain_av(prev)

            # ---------- phase 3: output projection (wide, lb-quads) ------
            with tc.tile_pool(name=f"yps{rep}", bufs=2, space="PSUM") as y_ps:
                yall = ysb.tile([128, 8 * DIM], f32, name="t", tag="yall")
                if _PH < 3:
                    nc.vector.memset(yall[:], 0.0)
                for lq in range(2 if _PH >= 3 else 0):
                    yp = y_ps.tile([128, 2048], f32, name="t", tag="yps")
                    for li in range(4):
                        lb = 4 * lq + li
                        for c in range(4):
                            nc.tensor.matmul(
                                yp[:, li * 512:(li + 1) * 512],
                                outT[c][:, lb * 128:(lb + 1) * 128],
                                wp_sb[c][:],
                                start=(c == 0), stop=(c == 3))
                    nc.vector.tensor_copy(yall[:, lq * 2048:(lq + 1) * 2048], yp[:])
                nc.sync.dma_start(y[:], yall[:])

        # Hardware loop over reps: the static instruction stream is emitted
        # once and replayed, so per-rep cost is dynamic-execution only.
        with tc.For_i(0, _REPS):
            emit_body(0)

    nc.compile()
    return nc


def _rope_tables():
    inv_freq = 1.0 / (10000.0 ** (np.arange(0, HD, 2, dtype=np.float32) / HD))
    t = np.arange(L, dtype=np.float32)
    freqs = np.outer(t, inv_freq)                      # (L, 32)
    emb = np.concatenate([freqs, freqs], axis=-1)      # (L, 64)
    cos = np.cos(emb).T                                # (64, L)
    sin = np.sin(emb).T                                # (64, L)
    sign = np.where(np.arange(HD) < HD // 2, -1.0, 1.0)[:, None].astype(np.float32)
    sin_s = sin * sign
    cosT = np.tile(cos, (2, 2)).astype(np.float16)     # (128, 2L)
    sinT = np.tile(sin_s, (2, 2)).astype(np.float16)   # (128, 2L)
    return cosT, sinT


_NC = None


def _get_nc():
    global _NC
    if _NC is None:
        _NC = _build_nc()
    return _NC


def kernel(x, mask, w_qkv, w_proj):
    x = np.asarray(x, dtype=np.float32)
    mask = np.asarray(mask)
    w_qkv = np.asarray(w_qkv, dtype=np.float32)
    w_proj = np.asarray(w_proj, dtype=np.float32)

    nc = _get_nc()
    cosT, sinT = _rope_tables()

    wq = np.ascontiguousarray(w_qkv[:, :2 * DIM]).astype(np.float16)
    wv = np.ascontiguousarray(w_qkv[:, 2 * DIM:]).astype(np.float16)
    wp = w_proj.astype(np.float16)

    in_maps = []
    for b in range(NCORES):
        xTb = np.ascontiguousarray(x[b].T).astype(np.float16)      # (512, 1024)
        bias_b = np.where(mask[b].reshape(8, 128).T, 0.0, -1e9).astype(np.float32)
        in_maps.append({
            "xT": xTb, "wq": wq, "wv": wv, "wp": wp,
            "cosT": cosT, "sinT": sinT, "bias": bias_b,
        })

    from concourse.bass_utils import run_bass_kernel_spmd
    res = run_bass_kernel_spmd(nc, in_maps, core_ids=list(range(NCORES)))
    out = np.stack([
        res.results[c]["y"].reshape(128, 8, DIM).transpose(1, 0, 2).reshape(L, DIM)
        for c in range(NCORES)
    ], axis=0)
    return out.astype(np.float32)


# revision 11
# speedup vs baseline: 1.3137x; 1.0699x over previous
"""Multi-head self-attention (RoPE) Trainium2 Bass kernel.

Shards batch (B=8) across 8 NeuronCores, one batch element per core.
Per core: fused qkv projection (fp16 matmuls), RoPE, flash-style attention
(scores row-tiled per head pair, exp on ACT with fused scale+mask-bias,
AV with a fused ones-column block producing softmax denominators),
reciprocal-normalize, output projection.

Performance structure:
- The rep body sits inside a tc.For_i hardware loop: the NEFF replays one
  static instruction stream per rep, so steady-state per-rep cost is
  dynamic execution only (~PE moving-column roofline) instead of paying
  ~25-70us/instruction static dispatch for unrolled reps.
- Phase-2 emission interleaves pair p's score groups with pair p-1's AV
  groups at 4-matmul granularity so the PE queue never stalls on the ACT
  exp of the single-buffered scores PSUM.
- V tiles carry the ones-block on opposite partition halves for even/odd
  heads, so a head-pair's denominators land on complementary partition
  halves and normalization needs no cross-partition DMA.
- RoPE and PSUM->SBUF evacuation run as wide (2048-col) DVE ops over
  m-block pairs; the ones-block memset is hoisted out of the rep loop;
  the output DMA is a contiguous 128x16KB descriptor copy (host undoes
  the block-row layout).
"""
import os
import sys

# The kernel needs the 8 axon-tunneled NeuronCores visible to jax; a
# JAX_PLATFORMS=cpu pin (used by some harnesses for the reference) would
# hide them. Clear it before jax initializes through the concourse imports.
os.environ.pop("JAX_PLATFORMS", None)

sys.path.insert(0, "/opt/trn_rl_repo")

_REPS = int(os.environ.get("KREPS", "1"))
_PH = int(os.environ.get("KPHASES", "3"))

import numpy as np
from contextlib import ExitStack

import concourse.bass as bass
import concourse.tile as tile
from concourse import bacc, mybir
from concourse.tile import add_dep_helper

f32 = mybir.dt.float32
f16 = mybir.dt.float16
AF = mybir.ActivationFunctionType
ALU = mybir.AluOpType

B, L, DIM = 8, 1024, 512
NH, HD = 8, 64
SCALE = HD ** -0.5
NCORES = 8

# qkT slot layout: within a phase-1 group g (m-blocks 2g, 2g+1), the two
# wide RoPE adds write heads (4g, 4g+2) rows 0:64 and (4g+1, 4g+3) rows
# 64:128 as contiguous 2048-col spans. K-heads sit 8 slots later.
_WMAP = {0: 0, 2: 1, 1: 2, 3: 3}


def _slot_q(h):
    return (h // 4) * 4 + _WMAP[h % 4]


def _slot_k(h):
    return 8 + _slot_q(h)


def _build_nc():
    nc = bacc.Bacc("TRN2", target_bir_lowering=False, debug=False, enable_asserts=False)

    xT = nc.dram_tensor("xT", (DIM, L), f16, kind="ExternalInput")
    wq = nc.dram_tensor("wq", (DIM, 2 * DIM), f16, kind="ExternalInput")  # Q|K cols
    wv = nc.dram_tensor("wv", (DIM, DIM), f16, kind="ExternalInput")      # V cols
    wp = nc.dram_tensor("wp", (DIM, DIM), f16, kind="ExternalInput")
    cosT = nc.dram_tensor("cosT", (128, 2 * L), f16, kind="ExternalInput")
    sinT = nc.dram_tensor("sinT", (128, 2 * L), f16, kind="ExternalInput")
    bias = nc.dram_tensor("bias", (128, 8), f32, kind="ExternalInput")
    # y holds the block-row layout (p, lb*DIM+d) = out[lb*128+p, d]; the
    # host reshapes back. Keeps the per-rep output DMA fully contiguous.
    y = nc.dram_tensor("y", (128, 8 * DIM), f32, kind="ExternalOutput")

    with ExitStack() as ctx:
        tc = ctx.enter_context(tile.TileContext(nc))
        cst = ctx.enter_context(tc.tile_pool(name="cst", bufs=1))
        sc = ctx.enter_context(tc.tile_pool(name="sc", bufs=2))
        pTp = ctx.enter_context(tc.tile_pool(name="pTp", bufs=20))
        nrm = ctx.enter_context(tc.tile_pool(name="nrm", bufs=2))
        ysb = ctx.enter_context(tc.tile_pool(name="ysb", bufs=1))

        # ---- load inputs ----
        xT_all = cst.tile([128, 4 * L], f16, name="t", tag="xTall")
        wq_all = cst.tile([128, 4 * 2 * DIM], f16, name="t", tag="wqall")
        wv_all = cst.tile([128, 4 * DIM], f16, name="t", tag="wvall")
        wp_all = cst.tile([128, 4 * DIM], f16, name="t", tag="wpall")
        for big, dram, w in ((xT_all, xT, L), (wq_all, wq, 2 * DIM),
                             (wv_all, wv, DIM), (wp_all, wp, DIM)):
            nc.sync.dma_start(
                big[:].rearrange("p (kc w) -> p kc w", kc=4),
                dram[:].rearrange("(kc p) w -> p kc w", p=128))
        xT_sb = [xT_all[:, i * L:(i + 1) * L] for i in range(4)]
        wq_sb = [wq_all[:, i * 2 * DIM:(i + 1) * 2 * DIM] for i in range(4)]
        wv_sb = [wv_all[:, i * DIM:(i + 1) * DIM] for i in range(4)]
        wp_sb = [wp_all[:, i * DIM:(i + 1) * DIM] for i in range(4)]
        cos_sb = cst.tile([128, 2 * L], f16, name="t", tag="cos")
        sin_sb = cst.tile([128, 2 * L], f16, name="t", tag="sin")
        bias_sb = cst.tile([128, 8], f32, name="t", tag="bias")
        nc.sync.dma_start(cos_sb[:], cosT[:])
        nc.sync.dma_start(sin_sb[:], sinT[:])
        nc.sync.dma_start(bias_sb[:], bias[:])

        # Q|K heads, one 16-slot tile; slot order per _slot_q/_slot_k.
        qkT = cst.tile([64, 16 * L], f16, name="t", tag="qkT")

        def qk_ap(slot):
            return qkT[:, slot * L:(slot + 1) * L]

        # V (augmented): per lb block of 1024 cols, head h at h*128..h*128+128.
        # Even h: [V(0:64) | ones(64:128)]; odd h: [ones(0:64) | V(64:128)].
        # The single pre-loop memset seeds the ones; in-loop V copies only
        # touch the V columns.
        vaug = cst.tile([128, 8 * NH * 128], f16, name="t", tag="vaug")
        nc.vector.memset(vaug[:], 1.0)

        outT = [cst.tile([128, L], f16, name="t", tag=f"outT{c}") for c in range(4)]

        def emit_body(rep):
            # ---------- phase 1a: Q,K projection + RoPE (wide, m-pairs) ----
            with tc.tile_pool(name=f"qkps{rep}", bufs=2, space="PSUM") as qk_ps:
                for g in range(4):
                    ps = qk_ps.tile([128, 2048], f32, name="t", tag="qkps")
                    for mi in range(2):
                        m = 2 * g + mi
                        for kc in range(4):
                            for qb in range(2):
                                nc.tensor.matmul(
                                    ps[:, mi * 1024 + qb * 512:
                                       mi * 1024 + (qb + 1) * 512],
                                    wq_sb[kc][:, m * 128:(m + 1) * 128],
                                    xT_sb[kc][:, qb * 512:(qb + 1) * 512],
                                    start=(kc == 0), stop=(kc == 3))
                    qc = sc.tile([128, 2048], f16, name="t", tag="qc")
                    nc.vector.tensor_copy(qc[:], ps[:])
                    sw = sc.tile([128, 2048], f16, name="t", tag="sw")
                    for (do, so) in ((0, 32), (32, 0), (64, 96), (96, 64)):
                        nc.vector.tensor_copy(sw[do:do + 32, :], qc[so:so + 32, :])
                    q1 = sc.tile([128, 2048], f16, name="t", tag="q1")
                    nc.vector.tensor_mul(q1[:], qc[:], cos_sb[:])
                    q2 = sc.tile([128, 2048], f16, name="t", tag="q2")
                    nc.vector.tensor_mul(q2[:], sw[:], sin_sb[:])
                    # heads (4g, 4g+2) rows 0:64 -> slots 4s+0,1;
                    # heads (4g+1, 4g+3) rows 64:128 -> slots 4s+2,3
                    base = (g % 2) * 4 + (8 if g >= 2 else 0)
                    nc.vector.tensor_add(qkT[:, base * L:(base + 2) * L],
                                         q1[0:64, :], q2[0:64, :])
                    nc.vector.tensor_add(qkT[:, (base + 2) * L:(base + 4) * L],
                                         q1[64:128, :], q2[64:128, :])

            # ---------- phase 1b: V projection (wide, lb-quads) ----------
            with tc.tile_pool(name=f"vps{rep}", bufs=2, space="PSUM") as v_ps:
                for lq in range(2):
                    vps = v_ps.tile([128, 2048], f32, name="t", tag="vps")
                    for li in range(4):
                        lb = 4 * lq + li
                        for kc in range(4):
                            nc.tensor.matmul(
                                vps[:, li * 512:(li + 1) * 512],
                                xT_sb[kc][:, lb * 128:(lb + 1) * 128],
                                wv_sb[kc][:],
                                start=(kc == 0), stop=(kc == 3))
                    for li in range(4):
                        lb = 4 * lq + li
                        src = vps[:, li * 512:(li + 1) * 512].rearrange(
                            "p (g c2) -> p g c2", g=4)
                        dst = vaug[:, lb * 1024:(lb + 1) * 1024].rearrange(
                            "p (g c4) -> p g c4", g=4)
                        nc.vector.tensor_copy(dst[:, :, 0:64], src[:, :, 0:64])
                        nc.vector.tensor_copy(dst[:, :, 192:256], src[:, :, 64:128])

            # ---------- phase 2: attention (row-tiled 64x128) ----------
            # Interleaved: pair p's scores fill the PE queue between pair
            # p-1's AV groups, so PE never idles on the ACT exp.
            with tc.tile_pool(name=f"sps{rep}", bufs=2, space="PSUM") as s_ps, \
                 tc.tile_pool(name=f"avps{rep}", bufs=2, space="PSUM") as av_ps:

                def score_groups(p):
                    pts = {}

                    def grp(t, kb):
                        h = 2 * p + t
                        QTc, KTc = qk_ap(_slot_q(h)), qk_ap(_slot_k(h))
                        s = s_ps.tile([128, 1024], f32, name="t", tag="s")
                        for qb in range(2):
                            nc.tensor.matmul(
                                s[:, qb * 512:(qb + 1) * 512],
                                KTc[:, kb * 128:(kb + 1) * 128],
                                QTc[:, qb * 512:(qb + 1) * 512],
                                start=True, stop=True)
                        pt = pTp.tile([128, 1024], f16, name="t", tag="pT")
                        nc.scalar.activation(pt[:], s[:], AF.Exp,
                                             bias=bias_sb[:, kb:kb + 1],
                                             scale=SCALE)
                        pts[(t, kb)] = pt

                    return pts, [(t, kb) for t in range(2) for kb in range(8)], grp

                def av_groups(p, pts):
                    Xs = {}

                    def grp(t, kc2):
                        h = 2 * p + t
                        if kc2 == 0:
                            Xs[t] = av_ps.tile([128, L], f32, name="t", tag="avX")
                        X = Xs[t]
                        for kc in (2 * kc2, 2 * kc2 + 1):
                            pt = pts[(t, kc)]
                            va = vaug[:, kc * 1024 + h * 128:kc * 1024 + (h + 1) * 128]
                            for qb in range(2):
                                nc.tensor.matmul(
                                    X[:, qb * 512:(qb + 1) * 512], va,
                                    pt[:, qb * 512:(qb + 1) * 512],
                                    start=(kc == 0), stop=(kc == 7))

                    def norm_a():
                        # even head's denominators (rows 64:128) -> D rows 0:64
                        D = nrm.tile([128, L], f32, name="t", tag="D")
                        nc.vector.tensor_copy(D[0:64, :], Xs[0][64:128, :])
                        Xs["D"] = D

                    def norm_b():
                        D = Xs["D"]
                        # odd head's denominators (rows 0:64) -> D rows 64:128
                        nc.vector.tensor_copy(D[64:128, :], Xs[1][0:64, :])
                        R = nrm.tile([128, L], f32, name="t", tag="R")
                        nc.vector.reciprocal_approx_fast(R[:], D[:])
                        nc.vector.tensor_mul(outT[p][0:64, :], Xs[0][0:64, :],
                                             R[0:64, :])
                        nc.vector.tensor_mul(outT[p][64:128, :], Xs[1][64:128, :],
                                             R[64:128, :])

                    return ([(t, kc2) for t in range(2) for kc2 in range(4)],
                            grp, norm_a, norm_b)

                def drain_av(prev):
                    pp_, akeys, agrp, na, nb = prev
                    for ak in akeys:
                        agrp(*ak)
                        if ak == (0, 3):
                            na()
                    nb()

                prev = None
                for p in range(4 if _PH >= 2 else 0):
                    pts, skeys, sgrp = score_groups(p)
                    if prev is not None and _PH >= 3:
                        pp_, akeys, agrp, na, nb = prev
                        for i, ak in enumerate(akeys):
                            sgrp(*skeys[2 * i])
                            sgrp(*skeys[2 * i + 1])
                            agrp(*ak)
                            if ak == (0, 3):
                                na()
                        nb()
                    else:
                        for sk in skeys:
                            sgrp(*sk)
                    if _PH >= 3:
                        akeys, agrp, na, nb = av_groups(p, pts)
                        prev = (p, akeys, agrp, na, nb)
                if prev is not None and _PH >= 3:
                    drain_av(prev)

            # ---------- phase 3: output projection (wide, lb-quads) ------
            with tc.tile_pool(name=f"yps{rep}", bufs=2, space="PSUM") as y_ps:
                yall = ysb.tile([128, 8 * DIM], f32, name="t", tag="yall")
                if _PH < 3:
                    nc.vector.memset(yall[:], 0.0)
                for lq in range(2 if _PH >= 3 else 0):
                    yp = y_ps.tile([128, 2048], f32, name="t", tag="yps")
                    for li in range(4):
                        lb = 4 * lq + li
                        for c in range(4):
                            nc.tensor.matmul(
                                yp[:, li * 512:(li + 1) * 512],
                                outT[c][:, lb * 128:(lb + 1) * 128],
                                wp_sb[c][:],
                                start=(c == 0), stop=(c == 3))
                    nc.vector.tensor_copy(yall[:, lq * 2048:(lq + 1) * 2048], yp[:])
                nc.sync.dma_start(y[:], yall[:])

        # Hardware loop over reps: the static instruction stream is emitted
        # once and replayed, so per-rep cost is dynamic-execution only.
        with tc.For_i(0, _REPS, staggered_reset=True):
            emit_body(0)

    nc.compile()
    return nc


def _rope_tables():
    inv_freq = 1.0 / (10000.0 ** (np.arange(0, HD, 2, dtype=np.float32) / HD))
    t = np.arange(L, dtype=np.float32)
    freqs = np.outer(t, inv_freq)                      # (L, 32)
    emb = np.concatenate([freqs, freqs], axis=-1)      # (L, 64)
    cos = np.cos(emb).T                                # (64, L)
    sin = np.sin(emb).T                                # (64, L)
    sign = np.where(np.arange(HD) < HD // 2, -1.0, 1.0)[:, None].astype(np.float32)
    sin_s = sin * sign
    cosT = np.tile(cos, (2, 2)).astype(np.float16)     # (128, 2L)
    sinT = np.tile(sin_s, (2, 2)).astype(np.float16)   # (128, 2L)
    return cosT, sinT


_NC = None


def _get_nc():
    global _NC
    if _NC is None:
        _NC = _build_nc()
    return _NC


def kernel(x, mask, w_qkv, w_proj):
    x = np.asarray(x, dtype=np.float32)
    mask = np.asarray(mask)
    w_qkv = np.asarray(w_qkv, dtype=np.float32)
    w_proj = np.asarray(w_proj, dtype=np.float32)

    nc = _get_nc()
    cosT, sinT = _rope_tables()

    wq = np.ascontiguousarray(w_qkv[:, :2 * DIM]).astype(np.float16)
    wv = np.ascontiguousarray(w_qkv[:, 2 * DIM:]).astype(np.float16)
    wp = w_proj.astype(np.float16)

    in_maps = []
    for b in range(NCORES):
        xTb = np.ascontiguousarray(x[b].T).astype(np.float16)      # (512, 1024)
        bias_b = np.where(mask[b].reshape(8, 128).T, 0.0, -1e9).astype(np.float32)
        in_maps.append({
            "xT": xTb, "wq": wq, "wv": wv, "wp": wp,
            "cosT": cosT, "sinT": sinT, "bias": bias_b,
        })

    from concourse.bass_utils import run_bass_kernel_spmd
    res = run_bass_kernel_spmd(nc, in_maps, core_ids=list(range(NCORES)))
    out = np.stack([
        res.results[c]["y"].reshape(128, 8, DIM).transpose(1, 0, 2).reshape(L, DIM)
        for c in range(NCORES)
    ], axis=0)
    return out.astype(np.float32)
